# revision 68
# baseline (speedup 1.0000x reference)
"""Trainium2 Bass kernel for nn_AttentionPoolingTemporalEncoder.

Strategy (data-parallel over batch, 8 cores, 4 batch rows each):
  device:  h = relu(x @ Wp)          fp8 DoubleRow matmuls (2 MAC/cell/cyc);
                                     Wp prescaled x32 into fp8 normal range
           scores = h @ ((Wk qh)/sqrt(D) * SCORE_SCALE)   (folded on host)
           p = exp(scores/SCORE_SCALE + maskbias)  (no running max needed)
           U[h,:] = sum_s p[s,h] h[s,:] ; Z[h] = sum_s p[s,h]  (DR-paired)
  host:    pooled = (U/(32 Z)) @ Wv (+bv) per head; @Wo+bo; @W2+b2; LayerNorm.

Key device-side structure (per 1024-position chunk of a batch row):
  - 8 s-tiles of fp8-DR h matmuls accumulate in PSUM, relu'd to fp8 SBUF
  - ONE batched XBAR DMA transpose of the whole chunk's h (fp8 byte-pairs
    viewed as bf16 -- halves transpose bytes; the ~1.2us/instr cost is
    fixed, so batching 8 tiles amortizes it)
  - attention tail (scores via stride-2 fp8 slices of the pair-transpose,
    exp on ScalarE, U/Z DR-paired over two s-tiles) runs pipelined 3
    chunks behind, ramping down near the end so the drain overlaps
  - x chunks prefetched distance-3 on the sync HWDGE ring; the initial
    three loads are chained via 1-byte DVE copies so chunk 0 isn't
    time-shared across SDMA lanes; HAM keep-warm filler matmuls cover
    the final transpose wait; U/Z output stores go out on the gpsimd
    (SWDGE) ring because each store waits ~11us for its row's final U
    matmul and would head-of-line-block the x loads on the sync ring
"""

import sys
import threading

import numpy as np

sys.path.insert(0, "/opt/trn_rl_repo")

from contextlib import ExitStack

import concourse.tile as tile
from concourse import bacc, mybir
from concourse.bass_utils import run_bass_kernel_spmd


def _ensure_axon_ntff_hook_module():
    """Some images lack ``antenv.axon_hooks``; concourse imports it
    unconditionally when tracing is requested (e.g. via BASS_TRACE).
    Provide a minimal stand-in so that path degrades to no-trace
    instead of crashing."""
    try:
        from antenv import axon_hooks  # noqa: F401

        return
    except ImportError:
        pass
    import types

    mod = types.ModuleType("antenv.axon_hooks")
    mod._hook = None

    def set_axon_ntff_profile_hook(h):
        mod._hook = h

    def get_axon_ntff_profile_hook():
        return mod._hook

    mod.set_axon_ntff_profile_hook = set_axon_ntff_profile_hook
    mod.get_axon_ntff_profile_hook = get_axon_ntff_profile_hook
    sys.modules["antenv.axon_hooks"] = mod
    try:
        import antenv

        antenv.axon_hooks = mod
    except ImportError:
        pass


_ensure_axon_ntff_hook_module()

# Problem sizes (hardcoded per spec)
B, S, IN_DIM, E, H = 32, 4096, 1024, 512, 8
D = E // H
NCORES = 8
P = 128
# Device-side scaling: Wp is prescaled x32 (fp8 normal range) so device h =
# 32*h_true; wkq is scaled so device scores = SCORE_SCALE*scores_true, undone
# by the exp() activation's scale=1/SCORE_SCALE.
SCORE_SCALE = 512.0

_nc_cache = {}
_nc_lock = threading.Lock()


def build_nc(BL=B // NCORES, S_=S, I_=IN_DIM, has_bp=False, no_mask=False, trace_label=""):
    """Build + compile the per-core Bass program.

    BL: batch rows per core. S_: sequence length. I_: input dim.
    has_bp: emit the extra K=1 matmul adding the input-projection bias.
    """
    key = (BL, S_, I_, has_bp, no_mask)
    with _nc_lock:
        if key in _nc_cache:
            return _nc_cache[key]

    IC = I_ // P        # input-dim chunks
    EC = E // P         # embed-dim chunks
    S_TILES = S_ // P   # sequence tiles per batch row
    S_BLK = min(1024, S_)
    BLKS = S_ // S_BLK
    TPB = S_BLK // P    # s-tiles per DMA block

    f32 = mybir.dt.float32
    bf16 = mybir.dt.bfloat16
    fp8 = mybir.dt.float8e4
    DR = mybir.MatmulPerfMode.DoubleRow
    RELU = mybir.ActivationFunctionType.Relu
    EXP = mybir.ActivationFunctionType.Exp
    COPY = mybir.ActivationFunctionType.Copy

    nc = bacc.Bacc(
        "TRN2",
        target_bir_lowering=False,
        debug=False,
        enable_asserts=False,
        num_devices=NCORES,
    )

    # DRAM I/O (per-core shapes). Matmul operands are bf16 (host-cast):
    # halves HBM traffic and runs the PE at full rate.
    NCH_ = S_ // min(1024, S_)
    SC_ = min(1024, S_)
    # x packed per chunk: xt[b, cc, p, c, s] = x[b, cc*SC+s, c*128+p] so each
    # chunk load is one 8 KiB contiguous run per partition (full DMA line rate)
    xt = nc.dram_tensor(
        "xt", [BL, NCH_, P, IC * SC_], fp8, kind="ExternalInput"
    ).ap()
    wp = nc.dram_tensor("wp", [IC, P, E], fp8, kind="ExternalInput").ap()
    # wkq packed for pair-transposed h: wkq4[half*2+parity, k, h] =
    # wkq[2*(half*128+k)+parity, h]
    wkq = nc.dram_tensor("wkq", [4, P, H], fp8, kind="ExternalInput").ap()
    mb = nc.dram_tensor("mb", [BL, P, S_TILES], f32, kind="ExternalInput").ap()
    if has_bp:
        bp_d = nc.dram_tensor("bp", [1, E], bf16, kind="ExternalInput").ap()
    u_out = nc.dram_tensor("u_out", [BL, H, E], f32, kind="ExternalOutput").ap()
    z_out = nc.dram_tensor("z_out", [BL, H, 1], f32, kind="ExternalOutput").ap()

    with tile.TileContext(nc) as tc, ExitStack() as ctx:
        const = ctx.enter_context(tc.tile_pool(name="const", bufs=1))
        xp = ctx.enter_context(tc.tile_pool(name="xp", bufs=4))
        hp = ctx.enter_context(tc.tile_pool(name="hp", bufs=5))
        htp = ctx.enter_context(tc.tile_pool(name="htp", bufs=5))
        pp = ctx.enter_context(tc.tile_pool(name="pp", bufs=3))
        mbp = ctx.enter_context(tc.tile_pool(name="mbp", bufs=2))
        uzp = ctx.enter_context(tc.tile_pool(name="uzp", bufs=2))
        ps_h = ctx.enter_context(tc.tile_pool(name="ps_h", bufs=3, space="PSUM"))
        ps_s = ctx.enter_context(tc.tile_pool(name="ps_s", bufs=2, space="PSUM"))
        ps_u = ctx.enter_context(tc.tile_pool(name="ps_u", bufs=2, space="PSUM"))
        ps_z = ctx.enter_context(tc.tile_pool(name="ps_z", bufs=1, space="PSUM"))

        # Resident constants.  wp goes out on the scalar HWDGE ring so the
        # first x chunk (sync ring) isn't queued behind it at startup.
        wp_sb = const.tile([P, IC, E], fp8)
        nc.scalar.dma_start(wp_sb[:], wp.rearrange("c p e -> p c e"))
        wkq_sb = const.tile([P, 4, H], fp8)
        nc.scalar.dma_start(wkq_sb[:], wkq.rearrange("c p h -> p c h"))
        # ones for the Z matmul, padded pair layout for DoubleRow (pair step
        # must be a multiple of 16 elements)
        ones_t = const.tile([P, 2, 16], fp8)
        nc.gpsimd.memset(ones_t[:], 1.0)
        if has_bp:
            ones_row = const.tile([1, P], bf16)
            nc.gpsimd.memset(ones_row[:], 1.0)
            bp_sb = const.tile([1, E], bf16)
            nc.scalar.dma_start(bp_sb[:], bp_d[:])



        # Chunked x prefetch (1024 s = 8 tiles per chunk)
        SC = min(1024, S_)
        NCH = S_ // SC
        TPC = SC // P
        chunks = [(bb, cc) for bb in range(BL) for cc in range(NCH)]

        def load_chunk(idx):
            bb, cc = chunks[idx]
            xt_c = xp.tile([P, IC, SC], fp8, tag="xchunk")
            nc.sync.dma_start(xt_c[:], xt[bb, cc].rearrange("p (c s) -> p c s", c=IC))
            return xt_c

        # distance-3 prefetch, with the three initial loads CHAINED: a
        # 1-byte DVE copy from chunk i's tile into chunk i+1's tile forces
        # load i+1 to dispatch only after load i completes.  Without this
        # the SDMA lanes round-robin across all three 1 MB loads and chunk
        # 0 arrives ~3x late, leaving the PE cold at startup.
        bufq = []
        for i in range(min(3, len(chunks))):
            if i == 0:
                bufq.append(load_chunk(i))
            else:
                bb, cc2 = chunks[i]
                xt_c = xp.tile([P, IC, SC], fp8, tag="xchunk", name="xt_c")
                nc.vector.tensor_copy(xt_c[0:1, 0:1, 0:1], bufq[-1][0:1, 0:1, 0:1])
                nc.sync.dma_start(
                    xt_c[:], xt[bb, cc2].rearrange("p (c s) -> p c s", c=IC)
                )
                bufq.append(xt_c)
        chunk_idx = len(bufq) - 1

        # Per-row state (row = one batch element), pipelined across rows:
        # the flat chunk loop lets chunk c+1's h-matmuls overlap chunk c's
        # attention tail even across a row boundary.
        mb_rows = []
        for b in range(BL):
            mb_t = mbp.tile([P, S_TILES], f32, tag=f"mb{b}")
            nc.gpsimd.dma_start(mb_t[:], mb[b])
            mb_rows.append(mb_t)
        uz_rows = [None] * BL
        uz_rows[0] = (
            ps_u.tile([H, E], f32, tag="u_ps", name="u_ps"),
            ps_z.tile([H, 2], f32, tag="z_ps", name="z_ps"),
        )
        pending = []

        def emit_chunk_tails(pend):
            b_, cc_, h8_c_, ht_c_ = pend
            mb_t = mb_rows[b_]
            u_ps, z_ps = uz_rows[b_]
            # ht_c_ holds the pair-packed transpose of the whole chunk (bf16
            # container = 2 fp8 bytes): ht_c_[k, 2*j+half, s].u16 =
            # (h[s, 2k'], h[s, 2k'+1]) with k' = half*128 + k, tile j.
            ht_f8 = ht_c_[:].bitcast(fp8)  # [128, 2*TPC, 256]
            for jj in range(TPC // 2):
                # two s-tiles per iteration so U/Z matmuls DoubleRow-pair
                # over K = 256 sequence positions
                p2 = pp.tile([P, 2, 16], fp8, tag="p2")
                for g in range(2):
                    j = 2 * jj + g
                    t_ = cc_ * TPC + j
                    # scores[s,h] = sum_e h[s,e] wkq[e,h]: two DoubleRow
                    # matmuls, pairing the e-halves for each byte parity
                    sc_ps = ps_s.tile([P, H], f32)
                    for par in range(2):
                        nc.tensor.matmul(
                            sc_ps[:],
                            ht_f8[:, 2 * j : 2 * j + 2, par::2],
                            wkq_sb[:, par::2, :],
                            start=(par == 0),
                            stop=(par == 1),
                            perf_mode=DR,
                        )
                    # p = exp(scores/SCORE_SCALE + maskbias); maskbias = 0
                    # for unmasked, -1e4 for masked positions
                    nc.scalar.activation(
                        p2[:, g, 0:H], sc_ps[:], EXP, bias=mb_t[:, t_ : t_ + 1],
                        scale=1.0 / SCORE_SCALE,
                    )
                t0 = cc_ * TPC + 2 * jj
                nc.tensor.matmul(
                    u_ps[:],
                    p2[:, :, 0:H],
                    h8_c_[:, 2 * jj : 2 * jj + 2, :],
                    start=(t0 == 0),
                    stop=(t0 + 2 == S_TILES),
                    perf_mode=DR,
                    skip_group_check=True,
                )
                nc.tensor.matmul(
                    z_ps[:],
                    p2[:, :, 0:H],
                    ones_t[:, :, 0:2],
                    start=(t0 == 0),
                    stop=(t0 + 2 == S_TILES),
                    perf_mode=DR,
                    skip_group_check=True,
                )

        def finish_row(b_):
            u_ps, z_ps = uz_rows[b_]
            u_sb = uzp.tile([H, E], f32, tag="u_sb")
            z_sb = uzp.tile([H, 1], f32, tag="z_sb")
            nc.vector.tensor_copy(u_sb[:], u_ps[:])
            nc.vector.tensor_copy(z_sb[:], z_ps[:, 0:1])
            # outputs go out on the idle gpsimd (SWDGE) ring: a store waits
            # ~11us for its row's final U matmul, and on the sync ring that
            # wait head-of-line-blocks the x-chunk loads queued behind it
            nc.gpsimd.dma_start(u_out[b_], u_sb[:])
            nc.gpsimd.dma_start(z_out[b_], z_sb[:])

        for ci, (b, cc) in enumerate(chunks):
            if cc == 0 and uz_rows[b] is None:
                uz_rows[b] = (
                    ps_u.tile([H, E], f32, tag="u_ps", name="u_ps"),
                    ps_z.tile([H, 2], f32, tag="z_ps", name="z_ps"),
                )
            # consume the next x chunk; keep three loads in flight
            x_sb = bufq.pop(0)
            if chunk_idx + 1 < len(chunks):
                chunk_idx += 1
                bufq.append(load_chunk(chunk_idx))

            h8_c = hp.tile([P, TPC, E], fp8, tag="h8c")
            for j in range(TPC):
                # h = relu(x @ Wp): accumulate i-chunk PAIRS into PSUM via
                # fp8 DoubleRow (2 MACs/cell/cycle, K=256 per matmul)
                h_ps = ps_h.tile([P, E], f32)
                for cp in range(IC // 2):
                    nc.tensor.matmul(
                        h_ps[:],
                        x_sb[:, 2 * cp : 2 * cp + 2, j * P : (j + 1) * P],
                        wp_sb[:, 2 * cp : 2 * cp + 2, :],
                        start=(cp == 0),
                        stop=(cp == IC // 2 - 1) and not has_bp,
                        perf_mode=DR,
                    )
                if has_bp:
                    nc.tensor.matmul(
                        h_ps[:], ones_row[:], bp_sb[:], start=False, stop=True
                    )
                nc.scalar.activation(h8_c[:, j, :], h_ps[:], RELU)
                last_h_ps = h_ps

            # ONE batched XBAR transpose for the whole chunk (the ~1.2us
            # per-instruction cost is fixed, so amortize it over 8 tiles).
            # Concatenating tiles along the input free dim concatenates
            # output rows: row r = j*256 + half*128 + k.
            ht_c = htp.tile([P, 2 * TPC, P], bf16, tag="htc")
            nc.sync.dma_start_transpose(ht_c[:], h8_c[:].bitcast(bf16))

            if ci == len(chunks) - 1:
                # End-phase HAM keep-warm + latency cover: these
                # dependency-free matmuls sit in the tensor queue BETWEEN the
                # last h-matmuls and the final tails, chewing through the
                # ~3us the last transpose needs to land (instead of the PE
                # idling there and re-throttling to 1.2 GHz for the drain).
                for _ in range(22):
                    nc.tensor.matmul(
                        last_h_ps[0:16, :],
                        wp_sb[:, 0:2, 0:16],
                        wp_sb[:, 0:2, :],
                        start=True,
                        stop=True,
                        perf_mode=DR,
                        skip_group_check=True,
                    )

            pending.append((b, cc, h8_c, ht_c))
            # steady-state tail depth 3 (gives each transpose a full chunk
            # period to land); ramp down toward the end so the final tails
            # overlap the last h-matmuls instead of draining on an idle
            # (and HAM-throttled) PE.
            keep = min(3, len(chunks) - 1 - ci)
            while len(pending) > keep:
                pb, pcc = pending[0][0], pending[0][1]
                emit_chunk_tails(pending.pop(0))
                if pcc == NCH - 1:
                    # the row's U/Z accumulation is complete: store it
                    finish_row(pb)
        while pending:
            pb, pcc = pending[0][0], pending[0][1]
            emit_chunk_tails(pending.pop(0))
            if pcc == NCH - 1:
                finish_row(pb)

    nc.compile()
    with _nc_lock:
        _nc_cache[key] = nc
    return nc


def prepare_core_inputs(x, mask, Wp, wkq_scaled, bp=None):
    """Host-side packing for ONE core's shard.

    x: (BL, S, IN_DIM) fp32; mask: (BL, S) int; wkq_scaled: (E, H) fp32.
    """
    import ml_dtypes

    bf16 = ml_dtypes.bfloat16
    fp8 = ml_dtypes.float8_e4m3
    BL_, S_, I_ = x.shape
    IC = I_ // P
    EC = E // P
    # xt[b, c, i_in, s] = x[b, s, c*128+i_in].  fp8 e4m3 (TRN variant, max
    # 240): |x| <~ 6 so no clipping needed.
    SC_ = min(1024, S_)
    NCH_ = S_ // SC_
    xt = np.ascontiguousarray(
        x.reshape(BL_, NCH_, SC_, IC, P).transpose(0, 1, 4, 3, 2)
        .reshape(BL_, NCH_, P, IC * SC_)
    ).astype(fp8)
    # Wp prescaled x32 so fp8 weights sit in the normal range (std ~1);
    # h on device is 32*h_true, compensated on the host (U /= 32) and in
    # wkq (wkq_dev = wkq_true/32).
    wp = np.ascontiguousarray(Wp.reshape(IC, P, E) * np.float32(32.0)).astype(fp8)
    # wkq in device scale (wkq_true * SCORE_SCALE/32), packed to match the
    # pair-transposed h: wkq4[half*2+parity, k, h] = wkq[2*(half*128+k)+parity]
    wkq4 = wkq_scaled.reshape(2, P, 2, H).transpose(0, 2, 1, 3).reshape(4, P, H)
    wkq = np.ascontiguousarray(wkq4).astype(fp8)
    # additive mask bias packed [BL, P, S_TILES]: 0 where kept, -1e4 where
    # masked (exp(-1e4 + s) underflows to exactly 0)
    mb = np.ascontiguousarray(
        ((mask.astype(np.float32) - 1.0) * 1.0e4)
        .reshape(BL_, S_ // P, P)
        .transpose(0, 2, 1)
    ).astype(np.float32)
    m = {"xt": xt, "wp": wp, "wkq": wkq, "mb": mb}
    if bp is not None:
        # device h is 32*h_true, so the pre-relu bias must be 32*bp
        m["bp"] = (np.asarray(bp) * np.float32(32.0)).astype(bf16).reshape(1, E)
    return m


def kernel(
    x, mask, query, Wp, bp, Wq, bq, Wk, bk, Wv, bv, Wo, bo, W2, b2, gamma, beta,
    _trace=False,
):
    x = np.asarray(x)
    mask = np.asarray(mask)
    BL = B // NCORES

    # Host-side folds (all tiny)
    qh = (np.asarray(query, np.float64) @ np.asarray(Wq, np.float64)
          + np.asarray(bq, np.float64)).reshape(H, D)
    wkq_scaled = np.einsum(
        "ehd,hd->eh",
        np.asarray(Wk, np.float64).reshape(E, H, D),
        qh,
    ) / np.sqrt(D)

    has_bp = bool(np.any(np.asarray(bp)))
    nc = build_nc(has_bp=has_bp)

    in_maps = []
    for c in range(NCORES):
        sl = slice(c * BL, (c + 1) * BL)
        in_maps.append(
            prepare_core_inputs(
                x[sl], mask[sl], np.asarray(Wp),
                (wkq_scaled * (SCORE_SCALE / 32.0)).astype(np.float32),
                bp=np.asarray(bp) if has_bp else None,
            )
        )

    res = run_bass_kernel_spmd(
        nc, in_maps, core_ids=list(range(NCORES)), trace=_trace
    )
    U = np.concatenate([r["u_out"] for r in res.results], axis=0)  # (B, H, E)
    Z = np.concatenate([r["z_out"] for r in res.results], axis=0)[..., :1]  # (B, H, 1)

    # Host epilogue in float64 (device h was 32*h_true -> U is 32*U_true)
    pooledH = U.astype(np.float64) / (32.0 * Z.astype(np.float64))  # (B, H, E)
    Wv64 = np.asarray(Wv, np.float64).reshape(E, H, D)
    pooled = np.einsum("bhe,ehd->bhd", pooledH, Wv64).reshape(B, E)
    pooled += np.asarray(bv, np.float64)
    pooled = pooled @ np.asarray(Wo, np.float64) + np.asarray(bo, np.float64)
    out = pooled @ np.asarray(W2, np.float64) + np.asarray(b2, np.float64)
    mu = out.mean(-1, keepdims=True)
    var = out.var(-1, keepdims=True)
    out = (out - mu) / np.sqrt(var + 1e-5) * np.asarray(gamma, np.float64) + np.asarray(
        beta, np.float64
    )
    out_f32 = out.astype(np.float32)
    if _trace:
        return out_f32, res
    return out_f32



# revision 69
# speedup vs baseline: 1.0909x; 1.0909x over previous
"""Trainium2 Bass kernel for nn_AttentionPoolingTemporalEncoder.

Strategy (data-parallel over batch, 8 cores, 4 batch rows each):
  device:  h = relu(x @ Wp)          fp8 DoubleRow matmuls (2 MAC/cell/cyc);
                                     Wp prescaled x32 into fp8 normal range
           scores = h @ ((Wk qh)/sqrt(D) * SCORE_SCALE)   (folded on host)
           p = exp(scores/SCORE_SCALE + maskbias)  (no running max needed)
           U[h,:] = sum_s p[s,h] h[s,:] ; Z[h] = sum_s p[s,h]  (DR-paired)
  host:    pooled = (U/(32 Z)) @ Wv (+bv) per head; @Wo+bo; @W2+b2; LayerNorm.

Key device-side structure (per 1024-position chunk of a batch row):
  - 8 s-tiles of fp8-DR h matmuls accumulate in PSUM, relu'd to fp8 SBUF
  - ONE batched XBAR DMA transpose of the whole chunk's h (fp8 byte-pairs
    viewed as bf16 -- halves transpose bytes; the ~1.2us/instr cost is
    fixed, so batching 8 tiles amortizes it)
  - attention tail (scores via stride-2 fp8 slices of the pair-transpose,
    exp on ScalarE, U/Z DR-paired over two s-tiles) runs pipelined 3
    chunks behind, ramping down near the end so the drain overlaps
  - x chunks prefetched distance-3 on the sync HWDGE ring; the initial
    three loads are chained via 1-byte DVE copies so chunk 0 isn't
    time-shared across SDMA lanes; HAM keep-warm filler matmuls cover
    the final transpose wait; U/Z output stores go out on the gpsimd
    (SWDGE) ring because each store waits ~11us for its row's final U
    matmul and would head-of-line-block the x loads on the sync ring
"""

import sys
import threading

import numpy as np

sys.path.insert(0, "/opt/trn_rl_repo")

from contextlib import ExitStack

import concourse.tile as tile
from concourse import bacc, mybir
from concourse.bass_utils import run_bass_kernel_spmd


def _ensure_axon_ntff_hook_module():
    """Some images lack ``antenv.axon_hooks``; concourse imports it
    unconditionally when tracing is requested (e.g. via BASS_TRACE).
    Provide a minimal stand-in so that path degrades to no-trace
    instead of crashing."""
    try:
        from antenv import axon_hooks  # noqa: F401

        return
    except ImportError:
        pass
    import types

    mod = types.ModuleType("antenv.axon_hooks")
    mod._hook = None

    def set_axon_ntff_profile_hook(h):
        mod._hook = h

    def get_axon_ntff_profile_hook():
        return mod._hook

    mod.set_axon_ntff_profile_hook = set_axon_ntff_profile_hook
    mod.get_axon_ntff_profile_hook = get_axon_ntff_profile_hook
    sys.modules["antenv.axon_hooks"] = mod
    try:
        import antenv

        antenv.axon_hooks = mod
    except ImportError:
        pass


_ensure_axon_ntff_hook_module()

# Problem sizes (hardcoded per spec)
B, S, IN_DIM, E, H = 32, 4096, 1024, 512, 8
D = E // H
NCORES = 8
P = 128
# Device-side scaling: Wp is prescaled x32 (fp8 normal range) so device h =
# 32*h_true; wkq is scaled so device scores = SCORE_SCALE*scores_true, undone
# by the exp() activation's scale=1/SCORE_SCALE.
SCORE_SCALE = 512.0

_nc_cache = {}
_nc_lock = threading.Lock()


def build_nc(BL=B // NCORES, S_=S, I_=IN_DIM, has_bp=False, no_mask=False, trace_label=""):
    """Build + compile the per-core Bass program.

    BL: batch rows per core. S_: sequence length. I_: input dim.
    has_bp: emit the extra K=1 matmul adding the input-projection bias.
    """
    key = (BL, S_, I_, has_bp, no_mask)
    with _nc_lock:
        if key in _nc_cache:
            return _nc_cache[key]

    IC = I_ // P        # input-dim chunks
    EC = E // P         # embed-dim chunks
    S_TILES = S_ // P   # sequence tiles per batch row
    S_BLK = min(1024, S_)
    BLKS = S_ // S_BLK
    TPB = S_BLK // P    # s-tiles per DMA block

    f32 = mybir.dt.float32
    bf16 = mybir.dt.bfloat16
    fp8 = mybir.dt.float8e4
    DR = mybir.MatmulPerfMode.DoubleRow
    RELU = mybir.ActivationFunctionType.Relu
    EXP = mybir.ActivationFunctionType.Exp
    COPY = mybir.ActivationFunctionType.Copy

    nc = bacc.Bacc(
        "TRN2",
        target_bir_lowering=False,
        debug=False,
        enable_asserts=False,
        num_devices=NCORES,
    )

    # DRAM I/O (per-core shapes). Matmul operands are bf16 (host-cast):
    # halves HBM traffic and runs the PE at full rate.
    NCH_ = S_ // min(1024, S_)
    SC_ = min(1024, S_)
    # x packed per chunk: xt[b, cc, p, c, s] = x[b, cc*SC+s, c*128+p] so each
    # chunk load is one 8 KiB contiguous run per partition (full DMA line rate)
    xt = nc.dram_tensor(
        "xt", [BL, NCH_, P, IC * SC_], fp8, kind="ExternalInput"
    ).ap()
    wp = nc.dram_tensor("wp", [IC, P, E], fp8, kind="ExternalInput").ap()
    # wkq packed for pair-transposed h: wkq4[half*2+parity, k, h] =
    # wkq[2*(half*128+k)+parity, h]
    wkq = nc.dram_tensor("wkq", [4, P, H], fp8, kind="ExternalInput").ap()
    mb = nc.dram_tensor("mb", [BL, P, S_TILES], f32, kind="ExternalInput").ap()
    if has_bp:
        bp_d = nc.dram_tensor("bp", [1, E], bf16, kind="ExternalInput").ap()
    u_out = nc.dram_tensor("u_out", [BL, H, E], f32, kind="ExternalOutput").ap()
    z_out = nc.dram_tensor("z_out", [BL, H, 1], f32, kind="ExternalOutput").ap()

    with tile.TileContext(nc) as tc, ExitStack() as ctx:
        const = ctx.enter_context(tc.tile_pool(name="const", bufs=1))
        xp = ctx.enter_context(tc.tile_pool(name="xp", bufs=4))
        hp = ctx.enter_context(tc.tile_pool(name="hp", bufs=5))
        htp = ctx.enter_context(tc.tile_pool(name="htp", bufs=5))
        pp = ctx.enter_context(tc.tile_pool(name="pp", bufs=3))
        mbp = ctx.enter_context(tc.tile_pool(name="mbp", bufs=2))
        uzp = ctx.enter_context(tc.tile_pool(name="uzp", bufs=2))
        ps_h = ctx.enter_context(tc.tile_pool(name="ps_h", bufs=3, space="PSUM"))
        ps_s = ctx.enter_context(tc.tile_pool(name="ps_s", bufs=2, space="PSUM"))
        ps_u = ctx.enter_context(tc.tile_pool(name="ps_u", bufs=2, space="PSUM"))
        ps_z = ctx.enter_context(tc.tile_pool(name="ps_z", bufs=1, space="PSUM"))

        # Resident constants.  wp goes out on the scalar HWDGE ring so the
        # first x chunk (sync ring) isn't queued behind it at startup.
        wp_sb = const.tile([P, IC, E], fp8)
        nc.scalar.dma_start(wp_sb[:], wp.rearrange("c p e -> p c e"))
        wkq_sb = const.tile([P, 4, H], fp8)
        nc.scalar.dma_start(wkq_sb[:], wkq.rearrange("c p h -> p c h"))
        # ones for the Z matmul, padded pair layout for DoubleRow (pair step
        # must be a multiple of 16 elements)
        ones_t = const.tile([P, 2, 16], fp8)
        nc.gpsimd.memset(ones_t[:], 1.0)
        if has_bp:
            ones_row = const.tile([1, P], bf16)
            nc.gpsimd.memset(ones_row[:], 1.0)
            bp_sb = const.tile([1, E], bf16)
            nc.scalar.dma_start(bp_sb[:], bp_d[:])



        # Chunked x prefetch (1024 s = 8 tiles per chunk)
        SC = min(1024, S_)
        NCH = S_ // SC
        TPC = SC // P
        chunks = [(bb, cc) for bb in range(BL) for cc in range(NCH)]

        def load_chunk(idx):
            bb, cc = chunks[idx]
            xt_c = xp.tile([P, IC, SC], fp8, tag="xchunk")
            nc.sync.dma_start(xt_c[:], xt[bb, cc].rearrange("p (c s) -> p c s", c=IC))
            return xt_c

        # distance-3 prefetch, with the three initial loads CHAINED: a
        # 1-byte DVE copy from chunk i's tile into chunk i+1's tile forces
        # load i+1 to dispatch only after load i completes.  Without this
        # the SDMA lanes round-robin across all three 1 MB loads and chunk
        # 0 arrives ~3x late, leaving the PE cold at startup.
        bufq = []
        for i in range(min(3, len(chunks))):
            if i == 0:
                bufq.append(load_chunk(i))
            else:
                bb, cc2 = chunks[i]
                xt_c = xp.tile([P, IC, SC], fp8, tag="xchunk", name="xt_c")
                nc.vector.tensor_copy(xt_c[0:1, 0:1, 0:1], bufq[-1][0:1, 0:1, 0:1])
                nc.sync.dma_start(
                    xt_c[:], xt[bb, cc2].rearrange("p (c s) -> p c s", c=IC)
                )
                bufq.append(xt_c)
        chunk_idx = len(bufq) - 1

        # Per-row state (row = one batch element), pipelined across rows:
        # the flat chunk loop lets chunk c+1's h-matmuls overlap chunk c's
        # attention tail even across a row boundary.
        mb_rows = []
        for b in range(BL):
            mb_t = mbp.tile([P, S_TILES], f32, tag=f"mb{b}")
            nc.gpsimd.dma_start(mb_t[:], mb[b])
            mb_rows.append(mb_t)
        uz_rows = [None] * BL
        uz_rows[0] = (
            ps_u.tile([H, E], f32, tag="u_ps", name="u_ps"),
            ps_z.tile([H, 2], f32, tag="z_ps", name="z_ps"),
        )
        pending = []

        def emit_chunk_tails(pend):
            b_, cc_, h8_c_, ht_c_ = pend
            mb_t = mb_rows[b_]
            u_ps, z_ps = uz_rows[b_]
            # ht_c_ holds the pair-packed transpose of the whole chunk (bf16
            # container = 2 fp8 bytes): ht_c_[k, 2*j+half, s].u16 =
            # (h[s, 2k'], h[s, 2k'+1]) with k' = half*128 + k, tile j.
            ht_f8 = ht_c_[:].bitcast(fp8)  # [128, 2*TPC, 256]
            for jj in range(TPC // 2):
                # two s-tiles per iteration so U/Z matmuls DoubleRow-pair
                # over K = 256 sequence positions
                p2 = pp.tile([P, 2, 16], fp8, tag="p2")
                for g in range(2):
                    j = 2 * jj + g
                    t_ = cc_ * TPC + j
                    # scores[s,h] = sum_e h[s,e] wkq[e,h]: two DoubleRow
                    # matmuls, pairing the e-halves for each byte parity
                    sc_ps = ps_s.tile([P, H], f32)
                    for par in range(2):
                        nc.tensor.matmul(
                            sc_ps[:],
                            ht_f8[:, 2 * j : 2 * j + 2, par::2],
                            wkq_sb[:, par::2, :],
                            start=(par == 0),
                            stop=(par == 1),
                            perf_mode=DR,
                        )
                    # p = exp(scores/SCORE_SCALE + maskbias); maskbias = 0
                    # for unmasked, -1e4 for masked positions
                    nc.scalar.activation(
                        p2[:, g, 0:H], sc_ps[:], EXP, bias=mb_t[:, t_ : t_ + 1],
                        scale=1.0 / SCORE_SCALE,
                    )
                t0 = cc_ * TPC + 2 * jj
                nc.tensor.matmul(
                    u_ps[:],
                    p2[:, :, 0:H],
                    h8_c_[:, 2 * jj : 2 * jj + 2, :],
                    start=(t0 == 0),
                    stop=(t0 + 2 == S_TILES),
                    perf_mode=DR,
                    skip_group_check=True,
                )
                nc.tensor.matmul(
                    z_ps[:],
                    p2[:, :, 0:H],
                    ones_t[:, :, 0:2],
                    start=(t0 == 0),
                    stop=(t0 + 2 == S_TILES),
                    perf_mode=DR,
                    skip_group_check=True,
                )

        def finish_row(b_):
            u_ps, z_ps = uz_rows[b_]
            u_sb = uzp.tile([H, E], f32, tag="u_sb")
            z_sb = uzp.tile([H, 1], f32, tag="z_sb")
            nc.vector.tensor_copy(u_sb[:], u_ps[:])
            nc.vector.tensor_copy(z_sb[:], z_ps[:, 0:1])
            # outputs go out on the idle gpsimd (SWDGE) ring: a store waits
            # ~11us for its row's final U matmul, and on the sync ring that
            # wait head-of-line-blocks the x-chunk loads queued behind it
            nc.gpsimd.dma_start(u_out[b_], u_sb[:])
            nc.gpsimd.dma_start(z_out[b_], z_sb[:])

        for ci, (b, cc) in enumerate(chunks):
            if cc == 0 and uz_rows[b] is None:
                uz_rows[b] = (
                    ps_u.tile([H, E], f32, tag="u_ps", name="u_ps"),
                    ps_z.tile([H, 2], f32, tag="z_ps", name="z_ps"),
                )
            # consume the next x chunk; keep three loads in flight
            x_sb = bufq.pop(0)
            if chunk_idx + 1 < len(chunks):
                chunk_idx += 1
                bufq.append(load_chunk(chunk_idx))

            h8_c = hp.tile([P, TPC, E], fp8, tag="h8c")
            for j in range(TPC):
                # h = relu(x @ Wp): accumulate i-chunk PAIRS into PSUM via
                # fp8 DoubleRow (2 MACs/cell/cycle, K=256 per matmul)
                h_ps = ps_h.tile([P, E], f32)
                for cp in range(IC // 2):
                    nc.tensor.matmul(
                        h_ps[:],
                        x_sb[:, 2 * cp : 2 * cp + 2, j * P : (j + 1) * P],
                        wp_sb[:, 2 * cp : 2 * cp + 2, :],
                        start=(cp == 0),
                        stop=(cp == IC // 2 - 1) and not has_bp,
                        perf_mode=DR,
                    )
                if has_bp:
                    nc.tensor.matmul(
                        h_ps[:], ones_row[:], bp_sb[:], start=False, stop=True
                    )
                nc.scalar.activation(h8_c[:, j, :], h_ps[:], RELU)
                last_h_ps = h_ps

            # ONE batched XBAR transpose for the whole chunk (the ~1.2us
            # per-instruction cost is fixed, so amortize it over 8 tiles).
            # Concatenating tiles along the input free dim concatenates
            # output rows: row r = j*256 + half*128 + k.
            ht_c = htp.tile([P, 2 * TPC, P], bf16, tag="htc")
            nc.sync.dma_start_transpose(ht_c[:], h8_c[:].bitcast(bf16))

            pending.append((b, cc, h8_c, ht_c))
            # steady-state tail depth 3 (gives each transpose a full chunk
            # period to land); ramp down toward the end so the final tails
            # overlap the last h-matmuls instead of draining on an idle
            # (and HAM-throttled) PE.
            keep = max(1, min(3, len(chunks) - 1 - ci))
            while len(pending) > keep:
                pb, pcc = pending[0][0], pending[0][1]
                emit_chunk_tails(pending.pop(0))
                if pcc == NCH - 1:
                    # the row's U/Z accumulation is complete: store it
                    finish_row(pb)
        # End-phase HAM keep-warm + latency cover: these dependency-free
        # matmuls sit in the tensor queue between the second-to-last tails
        # and the LAST chunk's tails, chewing through the ~3us its transpose
        # needs to land (instead of the PE idling and re-throttling).
        for _ in range(22):
            nc.tensor.matmul(
                last_h_ps[0:16, :],
                wp_sb[:, 0:2, 0:16],
                wp_sb[:, 0:2, :],
                start=True,
                stop=True,
                perf_mode=DR,
                skip_group_check=True,
            )
        while pending:
            pb, pcc = pending[0][0], pending[0][1]
            emit_chunk_tails(pending.pop(0))
            if pcc == NCH - 1:
                finish_row(pb)

    nc.compile()
    with _nc_lock:
        _nc_cache[key] = nc
    return nc


def prepare_core_inputs(x, mask, Wp, wkq_scaled, bp=None):
    """Host-side packing for ONE core's shard.

    x: (BL, S, IN_DIM) fp32; mask: (BL, S) int; wkq_scaled: (E, H) fp32.
    """
    import ml_dtypes

    bf16 = ml_dtypes.bfloat16
    fp8 = ml_dtypes.float8_e4m3
    BL_, S_, I_ = x.shape
    IC = I_ // P
    EC = E // P
    # xt[b, c, i_in, s] = x[b, s, c*128+i_in].  fp8 e4m3 (TRN variant, max
    # 240): |x| <~ 6 so no clipping needed.
    SC_ = min(1024, S_)
    NCH_ = S_ // SC_
    xt = np.ascontiguousarray(
        x.reshape(BL_, NCH_, SC_, IC, P).transpose(0, 1, 4, 3, 2)
        .reshape(BL_, NCH_, P, IC * SC_)
    ).astype(fp8)
    # Wp prescaled x32 so fp8 weights sit in the normal range (std ~1);
    # h on device is 32*h_true, compensated on the host (U /= 32) and in
    # wkq (wkq_dev = wkq_true/32).
    wp = np.ascontiguousarray(Wp.reshape(IC, P, E) * np.float32(32.0)).astype(fp8)
    # wkq in device scale (wkq_true * SCORE_SCALE/32), packed to match the
    # pair-transposed h: wkq4[half*2+parity, k, h] = wkq[2*(half*128+k)+parity]
    wkq4 = wkq_scaled.reshape(2, P, 2, H).transpose(0, 2, 1, 3).reshape(4, P, H)
    wkq = np.ascontiguousarray(wkq4).astype(fp8)
    # additive mask bias packed [BL, P, S_TILES]: 0 where kept, -1e4 where
    # masked (exp(-1e4 + s) underflows to exactly 0)
    mb = np.ascontiguousarray(
        ((mask.astype(np.float32) - 1.0) * 1.0e4)
        .reshape(BL_, S_ // P, P)
        .transpose(0, 2, 1)
    ).astype(np.float32)
    m = {"xt": xt, "wp": wp, "wkq": wkq, "mb": mb}
    if bp is not None:
        # device h is 32*h_true, so the pre-relu bias must be 32*bp
        m["bp"] = (np.asarray(bp) * np.float32(32.0)).astype(bf16).reshape(1, E)
    return m


def kernel(
    x, mask, query, Wp, bp, Wq, bq, Wk, bk, Wv, bv, Wo, bo, W2, b2, gamma, beta,
    _trace=False,
):
    x = np.asarray(x)
    mask = np.asarray(mask)
    BL = B // NCORES

    # Host-side folds (all tiny)
    qh = (np.asarray(query, np.float64) @ np.asarray(Wq, np.float64)
          + np.asarray(bq, np.float64)).reshape(H, D)
    wkq_scaled = np.einsum(
        "ehd,hd->eh",
        np.asarray(Wk, np.float64).reshape(E, H, D),
        qh,
    ) / np.sqrt(D)

    has_bp = bool(np.any(np.asarray(bp)))
    nc = build_nc(has_bp=has_bp)

    in_maps = []
    for c in range(NCORES):
        sl = slice(c * BL, (c + 1) * BL)
        in_maps.append(
            prepare_core_inputs(
                x[sl], mask[sl], np.asarray(Wp),
                (wkq_scaled * (SCORE_SCALE / 32.0)).astype(np.float32),
                bp=np.asarray(bp) if has_bp else None,
            )
        )

    res = run_bass_kernel_spmd(
        nc, in_maps, core_ids=list(range(NCORES)), trace=_trace
    )
    U = np.concatenate([r["u_out"] for r in res.results], axis=0)  # (B, H, E)
    Z = np.concatenate([r["z_out"] for r in res.results], axis=0)[..., :1]  # (B, H, 1)

    # Host epilogue in float64 (device h was 32*h_true -> U is 32*U_true)
    pooledH = U.astype(np.float64) / (32.0 * Z.astype(np.float64))  # (B, H, E)
    Wv64 = np.asarray(Wv, np.float64).reshape(E, H, D)
    pooled = np.einsum("bhe,ehd->bhd", pooledH, Wv64).reshape(B, E)
    pooled += np.asarray(bv, np.float64)
    pooled = pooled @ np.asarray(Wo, np.float64) + np.asarray(bo, np.float64)
    out = pooled @ np.asarray(W2, np.float64) + np.asarray(b2, np.float64)
    mu = out.mean(-1, keepdims=True)
    var = out.var(-1, keepdims=True)
    out = (out - mu) / np.sqrt(var + 1e-5) * np.asarray(gamma, np.float64) + np.asarray(
        beta, np.float64
    )
    out_f32 = out.astype(np.float32)
    if _trace:
        return out_f32, res
    return out_f32

